# revision 1
# baseline (speedup 1.0000x reference)
"""Trainium2 Bass kernel for nn_DiscreteStateSpaceModel_77077483094247.

Math: the reference computes y = einsum('nij,ijk->nik', u, K) but only uses
y[:, -1, :], so the whole model collapses to

    out = (u_t[:,-1,:] @ W_in.T + b_in) @ (C @ A_d^1023 @ B_d) @ W_out.T + b_out

A_d = expm(-0.01*HiPPO) is lower triangular, so A_d^1023[:n,:n] =
(A_d[:n,:n])^1023 exactly, and the sub-diagonal coupling decays like the
tail eigenvalues exp(-10.23*(2k+1)): measured in float64 on the real
inputs, truncating to n=32 moves the output by ~1e-6 relative.  With
A := A_d[:32,:32], C1 := C[:,:32], Btop := B_d[:32,:]:

    wb  = u_last @ W_in^T @ C1 + b_in^T C1          [2, 32]
    v   = wb @ A^1023                               [2, 32]
    out = v @ (Btop @ W_out^T) + b_out              [2, 512]

A^1023 via 9 squarings with a running product: S_{k+1} = S_k^2,
Pacc_{k+1} = S_k Pacc_k (Pacc_1 = A), G = S_9 Pacc_9 applied directly to
the 2-row vector (z = S_9^T wb^T, v^T = Pacc_9^T z).  Each chain
iteration is ONE fp32 PE matmul [32x64] (fp32 runs as LOW|HIGH 2-pass on
the PE); the S^T needed as the next lhsT comes from the DVE's 32x32
StreamTranspose, so the PE never transposes.  Everything off the
squaring chain runs in bf16 (single pass, ~1.8x faster than f32r
thanks to halved LDWEIGHTS/stream bytes; squaring in reduced precision
diverges - measured 1.7e-1 in f32r - but each linear factor only costs
~2^-9 relative): the weight images (W_in^T|u_last^T, W_out^T, Btop^T|C1,
biases) ship as bf16 halving the ~1MB DMA (descriptor-paced, ~5.6us for
1.1MB fp32) so the input projection lands in the chain's gaps, and the
tail operands (S_9|Pacc_9, wb^T, z, v^T, D) are bf16-rounded on the
PSUM->SBUF copies.  Total numeric cost vs the fp32 reference: 5.9e-3
relative (fixed inputs, deterministic) against the 2e-2 harness gate,
host-simulated at 5.7e-3 before committing.

Scheduling: the Tile list-scheduler trusts an optimistic DMA model, so
DMA-gated filler matmuls are pinned with tc.tile_wait_until() onto the
model's time axis; they then interleave into the chain's ~540ns
PSUM-copy gaps without head-of-line blocking the in-order PE.  All
weight layout prep (transposes, 128-partition SBUF images, u_last^T
appended to the W_in^T image, merged small constants) happens host-side;
DMAs ride only the HW-DGE engines (sync/scalar) - gpsimd SW-DGE data
measured ~5us late.  End-of-kernel cost (~9us: full 254-semaphore
per-engine zeroing + barrier ladder) and ~3us DMA-launch startup are
fixed by the framework (measured 15.8us for a 6-instruction kernel).

Sharding: u_t over batch (2 rows/core); small matrices replicated; the
chain duplicated per core (per the spec hint).
"""

import numpy as np
from contextlib import ExitStack

from concourse import bacc, bass, mybir, tile
from concourse import bass_utils

B_SZ, SEQ, D_IN, H_DIM, D_OUT = 16, 1024, 512, 256, 512
N_CORES = 8
B_LOC = B_SZ // N_CORES  # 2 batch rows per core

F32 = mybir.dt.float32
F32R = mybir.dt.float32r
BF16 = mybir.dt.bfloat16
P = 128
NB = 32  # chain block size
H2 = D_OUT // 2


def _build():
    nc = bacc.Bacc("TRN2", target_bir_lowering=False, debug=False,
                   num_devices=N_CORES)

    WA = H_DIM + B_LOC  # wint block width incl. appended u_last^T cols
    a0096 = nc.dram_tensor("a0096", [NB, 3 * NB], F32, kind="ExternalInput")
    wint = nc.dram_tensor("wint", [P, 4 * WA], BF16, kind="ExternalInput")
    bc = nc.dram_tensor("bc", [P, 4 * NB], BF16, kind="ExternalInput")
    wot = nc.dram_tensor("wot", [P, 2 * D_OUT], BF16, kind="ExternalInput")
    smalls = nc.dram_tensor("smalls", [1, H_DIM + B_LOC + D_OUT], BF16,
                            kind="ExternalInput")
    out = nc.dram_tensor("out", [B_LOC, D_OUT], F32, kind="ExternalOutput")

    with tile.TileContext(nc) as tc, ExitStack() as ctx:
        const = ctx.enter_context(tc.tile_pool(name="const", bufs=1))
        work = ctx.enter_context(tc.tile_pool(name="work", bufs=1))
        psum = ctx.enter_context(
            tc.tile_pool(name="psum", bufs=1, space=bass.MemorySpace.PSUM))

        V = nc.vector
        MM = nc.tensor.matmul

        # ---- DMA loads (HW-DGE only); a00/a00t first on their engines ----
        xa = [work.tile([NB, 3 * NB], F32 if k < 9 else BF16,
                        tag=f"xa{k}", name=f"xa{k}")
              for k in range(10)]
        # one image [A | A | A^T]: S_0, (Pacc slot), T_0
        nc.sync.dma_start(xa[0][:], a0096.ap()[:, :])

        # pregathered SBUF images; wint (with u_last^T appended per ko
        # block) split in 4 quarters over the 2 HW-DGE engines
        wint_sb = const.tile([P, 4, WA], BF16, tag="wint")
        wint_fl = wint_sb[:].rearrange("p ko h -> p (ko h)")
        nc.sync.dma_start(wint_fl[:, 0:WA], wint.ap()[:, 0:WA])
        nc.scalar.dma_start(wint_fl[:, WA:2 * WA], wint.ap()[:, WA:2 * WA])
        nc.sync.dma_start(wint_fl[:, 2 * WA:3 * WA],
                          wint.ap()[:, 2 * WA:3 * WA])
        nc.scalar.dma_start(wint_fl[:, 3 * WA:4 * WA],
                            wint.ap()[:, 3 * WA:4 * WA])
        ult_sb = wint_sb[:, :, H_DIM:H_DIM + B_LOC]
        smalls_sb = const.tile([1, H_DIM + B_LOC + D_OUT], BF16, tag="smalls")
        nc.scalar.dma_start(smalls_sb[:], smalls.ap()[:, :])
        bin_sb = smalls_sb[0:1, 0:H_DIM]
        ones2_sb = smalls_sb[0:1, H_DIM:H_DIM + B_LOC]
        bout_sb = smalls_sb[0:1, H_DIM + B_LOC:H_DIM + B_LOC + D_OUT]
        bc_sb = const.tile([P, 2, 2 * NB], BF16, tag="bc")
        nc.sync.dma_start(
            bc_sb[:].rearrange("p hb f -> p (hb f)"), bc.ap()[:, :])
        btt_sb = bc_sb[:, :, 0:NB]
        c1_sb = bc_sb[:, :, NB:2 * NB]
        wot_sb = const.tile([P, 2, D_OUT], BF16, tag="wot")
        wot_fl = wot_sb[:].rearrange("p hb d -> p (hb d)")
        nc.sync.dma_start(wot_fl[:, 0:D_OUT], wot.ap()[:, 0:D_OUT])
        nc.scalar.dma_start(wot_fl[:, D_OUT:2 * D_OUT],
                            wot.ap()[:, D_OUT:2 * D_OUT])

        # Pacc_1 = A (SBUF->SBUF, early, off the critical path)
        V.tensor_copy(xa[1][:, NB:2 * NB], xa[0][:, NB:2 * NB])

        # ---- filler jobs (scheduler redistributes into chain gaps) -------
        g_sb = work.tile([P, 2, B_LOC], BF16, tag="g")
        rt0_sb = work.tile([NB, B_LOC], BF16, tag="rt0")
        z_sb = work.tile([NB, B_LOC], BF16, tag="z")
        vt_sb = work.tile([NB, B_LOC], BF16, tag="vt")
        d_sb = work.tile([NB, D_OUT], BF16, tag="d")
        g_ps = psum.tile([P, 2, B_LOC], F32, tag="g", bufs=1)
        w_ps = psum.tile([NB, B_LOC], F32, tag="small2", bufs=1)
        d_ps = psum.tile([NB, D_OUT], F32, tag="d", bufs=1)
        out_ps0 = psum.tile([B_LOC, H2], F32, tag="o0", bufs=1)
        out_ps1 = psum.tile([B_LOC, H2], F32, tag="o1", bufs=1)

        def g_seeds():
            with tc.tile_wait_until(0.0044):
                for hb in range(2):
                    MM(g_ps[:, hb, :], bin_sb[0:1, P * hb:P * (hb + 1)],
                       ones2_sb[0:1, :], start=True, stop=False)

        def g_acc(hb, ko, wait):
            def go():
                with tc.tile_wait_until(wait):
                    MM(g_ps[:, hb, :], wint_sb[:, ko, P * hb:P * (hb + 1)],
                       ult_sb[:, ko, :], start=False, stop=(ko == 3))
                    if ko == 3:
                        V.tensor_copy(g_sb[:, hb, :], g_ps[:, hb, :])
            return go

        def wt_job(hb):
            # wb^T = C1^T @ (W_in u^T + b_in x 1)
            def go():
                with tc.tile_wait_until(0.0078 + 0.0004 * hb):
                    MM(w_ps[:], c1_sb[:, hb, :], g_sb[:, hb, :],
                       start=(hb == 0), stop=(hb == 1))
                    if hb == 1:
                        V.tensor_copy(rt0_sb[:], w_ps[:])
            return go

        def seed_job():
            with tc.tile_wait_until(0.0048):
                MM(out_ps0[:], ones2_sb[0:1, :], bout_sb[0:1, 0:H2],
                   start=True, stop=False)
                MM(out_ps1[:], ones2_sb[0:1, :], bout_sb[0:1, H2:D_OUT],
                   start=True, stop=False)

        def d_job(ko):
            # D = Btop @ W_out^T   [32, 512]
            def go():
                with tc.tile_wait_until(0.0072 + 0.0006 * ko):
                    MM(d_ps[:], btt_sb[:, ko, :], wot_sb[:, ko, :],
                       start=(ko == 0), stop=(ko == 1))
                if ko == 1:
                    with tc.tile_wait_until(0.0088):
                        V.tensor_copy(d_sb[:, 0:H2], d_ps[:, 0:H2])
                    with tc.tile_wait_until(0.0098):
                        V.tensor_copy(d_sb[:, H2:D_OUT], d_ps[:, H2:D_OUT])
            return go

        g_order = [(0, 0), (1, 0), (0, 1), (1, 1),
                   (0, 2), (1, 2), (0, 3), (1, 3)]
        jobs = [g_seeds] + [
            g_acc(hb, ko, 0.0046 + 0.0004 * j)
            for j, (hb, ko) in enumerate(g_order)] + [
            wt_job(0), wt_job(1), seed_job, d_job(0), d_job(1)]
        gap_plan = [1, 1, 1, 1, 2, 2, 2, 2, 1]

        def emit_jobs(n):
            for _ in range(n):
                if jobs:
                    jobs.pop(0)()

        # ---- squaring chain ----------------------------------------------
        # iter 0: S_1 = A^2 (Pacc_1 = A copied above)
        ps = psum.tile([NB, 2 * NB], F32, tag="chain", bufs=3)
        MM(ps[:, 0:NB], xa[0][:, 2 * NB:3 * NB], xa[0][:, 0:NB],
           start=True, stop=True)
        V.tensor_copy(xa[1][:, 0:NB], ps[:, 0:NB])
        V.transpose(xa[1][:, 2 * NB:3 * NB], ps[:, 0:NB])
        emit_jobs(gap_plan[0])
        # iters 1..8: [S_{k+1} | Pacc_{k+1}] = S_k @ [S_k | Pacc_k]
        for k in range(1, 9):
            ps = psum.tile([NB, 2 * NB], F32, tag="chain", bufs=3)
            MM(ps[:, 0:2 * NB], xa[k][:, 2 * NB:3 * NB], xa[k][:, 0:2 * NB],
               start=True, stop=True)
            if k < 8:
                V.tensor_copy(xa[k + 1][:, 0:2 * NB], ps[:, 0:2 * NB])
                V.transpose(xa[k + 1][:, 2 * NB:3 * NB], ps[:, 0:NB])
            else:
                # split the final cast: z needs only S_9; P_9 rounds
                # in parallel with the z matmul
                V.tensor_copy(xa[9][:, 0:NB], ps[:, 0:NB])
                V.tensor_copy(xa[9][:, NB:2 * NB], ps[:, NB:2 * NB])
            emit_jobs(gap_plan[k])
        emit_jobs(len(jobs))

        # ---- tail: v^T = Pacc_9^T S_9^T wb^T; out = v @ D + bias ---------
        MM(w_ps[:], xa[9][:, 0:NB], rt0_sb[:],
           start=True, stop=True)
        V.tensor_copy(z_sb[:], w_ps[:])
        MM(w_ps[:], xa[9][:, NB:2 * NB], z_sb[:],
           start=True, stop=True)
        V.tensor_copy(vt_sb[:], w_ps[:])

        out_sb = work.tile([B_LOC, D_OUT], F32, tag="osb")
        MM(out_ps0[:], vt_sb[:], d_sb[:, 0:H2], start=False, stop=True)
        V.tensor_copy(out_sb[:, 0:H2], out_ps0[:])
        nc.sync.dma_start(out.ap()[:, 0:H2], out_sb[:, 0:H2])
        MM(out_ps1[:], vt_sb[:], d_sb[:, H2:D_OUT], start=False, stop=True)
        V.tensor_copy(out_sb[:, H2:D_OUT], out_ps1[:])
        nc.scalar.dma_start(out.ap()[:, H2:D_OUT], out_sb[:, H2:D_OUT])

    nc.compile()
    return nc


_NC_CACHE = {}


def _get_nc():
    if "nc" not in _NC_CACHE:
        _NC_CACHE["nc"] = _build()
    return _NC_CACHE["nc"]


def kernel(u_t, W_in, b_in, C, W_out, b_out, A_d, B_d, **run_kwargs):
    nc = _get_nc()
    u_t = np.asarray(u_t, dtype=np.float32)
    A_d = np.asarray(A_d, dtype=np.float32)
    def img(mat, groups):
        # [(g p), w] -> [p, (g w)] sbuf image, p=128
        m = np.ascontiguousarray(mat, dtype=np.float32)
        g, w = groups, m.shape[1]
        return np.ascontiguousarray(
            m.reshape(g, P, w).transpose(1, 0, 2).reshape(P, g * w))

    import ml_dtypes
    BF = ml_dtypes.bfloat16
    smalls = np.concatenate([
        np.asarray(b_in, dtype=np.float32),
        np.ones(B_LOC, dtype=np.float32),
        np.asarray(b_out, dtype=np.float32)])[None, :].astype(BF)
    A00 = A_d[0:NB, 0:NB]
    a0096 = np.concatenate([A00, A00, A00.T], axis=1)
    wint_img = img(np.asarray(W_in, dtype=np.float32).T, 4)  # [128, 4*256]
    bc = np.concatenate([
        img(np.asarray(B_d, dtype=np.float32)[0:NB, :].T, 2)
        .reshape(P, 2, NB),
        img(np.asarray(C, dtype=np.float32)[:, 0:NB], 2).reshape(P, 2, NB),
    ], axis=2).reshape(P, 4 * NB)
    shared = {
        "a0096": np.ascontiguousarray(a0096),
        "bc": np.ascontiguousarray(bc.astype(BF)),
        "wot": img(np.asarray(W_out, dtype=np.float32).T, 2).astype(BF),
        "smalls": np.ascontiguousarray(smalls),
    }
    in_maps = []
    for i in range(N_CORES):
        m = dict(shared)
        ult = img(u_t[i * B_LOC:(i + 1) * B_LOC, SEQ - 1, :].T, 4)
        m["wint"] = np.ascontiguousarray(np.concatenate([
            wint_img.reshape(P, 4, H_DIM),
            ult.reshape(P, 4, B_LOC)], axis=2).reshape(P, -1).astype(BF))
        in_maps.append(m)
    res = bass_utils.run_bass_kernel_spmd(
        nc, in_maps, core_ids=list(range(N_CORES)), **run_kwargs)
    out = np.concatenate([res.results[i]["out"] for i in range(N_CORES)],
                         axis=0)
    if run_kwargs:
        return out, res
    return out

